# revision 1
# baseline (speedup 1.0000x reference)
"""Trainium2 Bass kernel for a GPT-2 transformer layer (B=4, T=2048, C=1024, H=16).

Sharding: 8 cores, one batch per core-pair; each core owns 1024 query tokens
(two 512-row blocks: an "early" block qbA and a "late" block qbB chosen so the
per-core causal attention work is balanced and the SPMD program is uniform).
No collectives: each core computes K/V for all 2048 tokens of its batch
(small redundancy), attention + MLP for its own rows only.

Causal structure (uniform across cores, causality in host-built mask data):
  qbA (query cols 0:512 of xq)  -> k-tiles 0..7,  additive mask on all 8
  qbB (query cols 512:1024)     -> k-tiles 0..15, additive mask on tiles 8..15
Host assignment (batch b = core//2, j = core%2):
  qbA = rows [j*512 : (j+1)*512),  qbB = rows [1024+j*512 : 1536+j*512)
qbA's needed keys are within [0:1024); qbB's keys [0:1024) are always fully
allowed (unmasked tiles) and keys [1024:2048) carry the mask. k-tiles beyond
the causal frontier are skipped entirely.

LayerNorms are folded into the matmuls: the device scales x rows by
rsqrt(var+eps) per token and two augmented contraction rows (mu*r, 1) paired
with host-folded weight rows (-colsum(g*W), ln_b@W + b) add the mean/bias
terms inside the same matmul accumulation.

Attention runs transposed (S^T[k,q] tiles): softmax denominators come from an
extra ones-column in the PV stationary; per-head 1/den is applied after PV
(gpsimd partition-broadcast + multiply). All matmuls are bf16 with f32 PSUM
accumulation.
"""

import numpy as np
import ml_dtypes

import concourse.bass as bass
import concourse.mybir as mybir
import concourse.tile as tile
from concourse import bacc
from concourse.bass import ts
from concourse.bass_utils import run_bass_kernel_spmd
from concourse.masks import make_identity

B, T, C, H = 4, 2048, 1024, 16
D = C // H          # 64
TQ = T // 2         # own query tokens per core = 1024
NCORES = 8
EPS = 1e-5
MASK_VAL = -1e30

F32 = mybir.dt.float32
BF16 = mybir.dt.bfloat16
AF = mybir.ActivationFunctionType

NT = T // 128        # 16 token tiles (all tokens)
NQ = TQ // 128       # 8 token tiles (own tokens)
NC8 = C // 128       # 8 c tiles
NF = 4 * C // 128    # 32 fc hidden tiles


def _ln_stats(nc, pool, x_t, ncols):
    """Per-partition mean/rsqrt stats of a [128, ncols] tile.
    Returns (r, m): r = rsqrt(var+eps), m = mu * r, both [128, 1] f32."""
    s = pool.tile([128, 1], F32, tag="ln_sum", name="ln_sum")
    ss = pool.tile([128, 1], F32, tag="ln_ssq", name="ln_ssq")
    trash = pool.tile([128, ncols], BF16, tag="ln_trash", name="ln_trash", bufs=1)
    nc.scalar.activation(trash[:], x_t[:], AF.Copy, accum_out=s[:])
    nc.scalar.activation(trash[:], x_t[:], AF.Square, accum_out=ss[:])
    mu = pool.tile([128, 1], F32, tag="ln_mu", name="ln_mu")
    nc.vector.tensor_scalar_mul(mu[:], s[:], 1.0 / ncols)
    ex2 = pool.tile([128, 1], F32, tag="ln_ex2", name="ln_ex2")
    nc.vector.tensor_scalar_mul(ex2[:], ss[:], 1.0 / ncols)
    var = pool.tile([128, 1], F32, tag="ln_var", name="ln_var")
    nc.vector.tensor_mul(var[:], mu[:], mu[:])
    nc.vector.tensor_sub(var[:], ex2[:], var[:])
    nc.vector.tensor_scalar_add(var[:], var[:], EPS)
    std = pool.tile([128, 1], F32, tag="ln_std", name="ln_std")
    nc.scalar.sqrt(std[:], var[:])
    r = pool.tile([128, 1], F32, tag="ln_r", name="ln_r")
    nc.vector.reciprocal(r[:], std[:])
    return mu, r


def _ln_transpose(nc, sp, psp, ident, src_tile, dstT, n_tiles, wk):
    """LayerNorm ((x-mu)*rsqrt) token-major [128, C] tiles into bf16 and
    PE-transpose into dstT (c-major, bf16), 4 token tiles per PSUM drain.
    src_tile: callable tt -> f32 AP."""
    for tt0 in range(0, n_tiles, 4):
        xs_ts = []
        for tt in range(tt0, tt0 + 4):
            x_t = src_tile(tt)
            mu, r = _ln_stats(nc, sp, x_t, C)
            xs_t = wk.tile([128, C], BF16, tag="xs_t", name="xs_t", bufs=5)
            nc.vector.tensor_scalar(xs_t[:], x_t[:], mu[:], r[:],
                                    mybir.AluOpType.subtract,
                                    mybir.AluOpType.mult)
            xs_ts.append(xs_t)
        for cc in range(NC8):
            pst4 = psp.tile([128, 4, 128], BF16, tag="tr", name="pst4")
            for i in range(4):
                nc.tensor.transpose(pst4[:, i, :],
                                    xs_ts[i][:, ts(cc, 128)], ident[:])
            nc.vector.tensor_copy(
                dstT[cc][:, tt0 * 128:(tt0 + 4) * 128], pst4[:])


def build_program(gelu_fn=None, loop_n=1, has_bias=False):
    nc = bacc.Bacc("TRN2", target_bir_lowering=False, debug=False)
    if gelu_fn is None:
        gelu_fn = AF.Gelu

    xb = nc.dram_tensor("xb", [T, C], F32, kind="ExternalInput")
    xq = nc.dram_tensor("xq", [TQ, C], F32, kind="ExternalInput")
    maskc = nc.dram_tensor("maskc", [T, 512], BF16, kind="ExternalInput")
    w1aug = nc.dram_tensor("w1aug", [C + 1, 3 * C], BF16, kind="ExternalInput")
    wpaug = nc.dram_tensor("wpaug", [C + 1, C], BF16, kind="ExternalInput")
    w2aug = nc.dram_tensor("w2aug", [C + 1, 4 * C], BF16, kind="ExternalInput")
    w3aug = nc.dram_tensor("w3aug", [4 * C + 1, C], BF16, kind="ExternalInput")
    out = nc.dram_tensor("out", [TQ, C], F32, kind="ExternalOutput")

    with tile.TileContext(nc) as tc:
        with (
            tc.tile_pool(name="glob", bufs=1) as pg,
            tc.tile_pool(name="stats", bufs=2) as sp,
            tc.tile_pool(name="psacc", bufs=4, space="PSUM") as psa,
            tc.tile_pool(name="pstr", bufs=2, space="PSUM") as psp,
        ):
            ident = pg.tile([128, 128], BF16, tag="ident", name="ident")
            make_identity(nc, ident[:])

            import contextlib
            loop_cm = tc.For_i(0, loop_n, 1) if loop_n > 1 else contextlib.nullcontext()
            with loop_cm, tc.tile_pool(name="p34", bufs=1) as p34:
                attnT = [p34.tile([128, TQ], BF16, tag=f"attnT{dt}",
                                  name=f"attnT{dt}") for dt in range(NC8)]

                with tc.tile_pool(name="att", bufs=1) as pa:
                    V_sb = [[pa.tile([128, 8, 65], BF16, tag=f"V{tt}_{hb}",
                                     name=f"V{tt}_{hb}") for hb in range(2)]
                            for tt in range(NT)]
                    mask_sb = [pa.tile([128, 2, 512], BF16, tag=f"mask{pp}",
                                       name=f"mask{pp}") for pp in range(NT // 2)]
                    for pp in range(NT // 2):
                        nc.sync.dma_start(
                            mask_sb[pp][:],
                            maskc[pp * 256:(pp + 1) * 256, :].rearrange(
                                "(i p) q -> p i q", p=128))

                    # ---------- Phase 1: LN1 + transpose (xb and xq) ----------
                    with tc.tile_pool(name="ph12", bufs=1) as p12, \
                         tc.tile_pool(name="w12", bufs=3) as wp, \
                         tc.tile_pool(name="wk12", bufs=2) as wk:
                        xsT = [p12.tile([128, T], BF16, tag=f"xsT{cc}",
                                        name=f"xsT{cc}") for cc in range(NC8)]
                        xqsT = [p12.tile([128, TQ], BF16, tag=f"xqsT{cc}",
                                         name=f"xqsT{cc}") for cc in range(NC8)]
                        ones_t = None
                        if has_bias:
                            ones_t = p12.tile([1, T], BF16, tag="ones_t",
                                              name="ones_t")
                            nc.vector.memset(ones_t[:, :], 1.0)

                        def _load_xb(tt):
                            t = wk.tile([128, C], F32, tag="xb_t", name="xb_t")
                            nc.sync.dma_start(t[:], xb[ts(tt, 128), :])
                            return t

                        def _load_xq(tt):
                            t = wk.tile([128, C], F32, tag="xb_t", name="xq_t")
                            nc.sync.dma_start(t[:], xq[ts(tt, 128), :])
                            return t

                        _ln_transpose(nc, sp, psp, ident, _load_xb,
                                      xsT, NT, wk)
                        _ln_transpose(nc, sp, psp, ident, _load_xq,
                                      xqsT, NQ, wk)

                        # ---------- Phase 2+3: QKV + attention, interleaved ----
                        def qkv_chain(dst, dst_slice, w_col0, n_blk,
                                      blk_src):
                            """One output column-block chain: 8 c-tiles (+ bias).
                            All 8 stationary tiles come in one strided DMA."""
                            w_t = wp.tile([128, NC8, 128], BF16, tag="w1_t",
                                          name="w1_t", bufs=3)
                            nc.sync.dma_start(
                                w_t[:],
                                w1aug[0:C, w_col0:w_col0 + 128].rearrange(
                                    "(cc p) n -> p cc n", p=128))
                            w_aug = None
                            if has_bias:
                                w_aug = wp.tile([1, 128], BF16, tag="w1_aug",
                                                name="w1_aug")
                                nc.sync.dma_start(
                                    w_aug[:], w1aug[C:C + 1, w_col0:w_col0 + 128])
                            for blk in range(n_blk):
                                ps = psa.tile([128, 512], F32, tag="acc",
                                              name="ps_qkv")
                                for cc in range(NC8):
                                    nc.tensor.matmul(ps[:], w_t[:, cc, :],
                                                     blk_src(cc, blk),
                                                     start=(cc == 0),
                                                     stop=(cc == NC8 - 1
                                                           and not has_bias))
                                if has_bias:
                                    nc.tensor.matmul(ps[:], w_aug[:],
                                                     ones_t[:, ts(blk, 512)],
                                                     start=False, stop=True)
                                nc.vector.tensor_copy(dst_slice(dst, blk), ps[:])

                        # V for all heads (token-major), per hd-block
                        def v_block(hb):
                            w_ts = []
                            for cc in range(NC8):
                                w_t = wp.tile([128, 512], BF16, tag="w1v_t",
                                              name="w1v_t", bufs=8)
                                nc.sync.dma_start(
                                    w_t[:],
                                    w1aug[ts(cc, 128),
                                          2 * C + hb * 512:2 * C + (hb + 1) * 512])
                                w_ts.append(w_t)
                            w_aug = None
                            if has_bias:
                                w_aug = wp.tile([1, 512], BF16, tag="w1v_aug",
                                                name="w1v_aug", bufs=2)
                                nc.sync.dma_start(
                                    w_aug[:],
                                    w1aug[C:C + 1,
                                          2 * C + hb * 512:2 * C + (hb + 1) * 512])
                            for tt in range(NT):
                                ps = psa.tile([128, 512], F32, tag="acc",
                                              name="ps_v")
                                for cc in range(NC8):
                                    nc.tensor.matmul(ps[:],
                                                     xsT[cc][:, ts(tt, 128)],
                                                     w_ts[cc][:],
                                                     start=(cc == 0),
                                                     stop=(cc == NC8 - 1
                                                           and not has_bias))
                                if has_bias:
                                    nc.tensor.matmul(ps[:],
                                                     ones_t[:, ts(tt, 128)],
                                                     w_aug[:],
                                                     start=False, stop=True)
                                vt = V_sb[tt][hb]
                                nc.vector.tensor_copy(
                                    vt[:, :, 0:64],
                                    ps[:].rearrange("p (h d) -> p h d", h=8))
                                nc.vector.memset(vt[:, :, 64:65], 1.0)

                        def attention_head(h, kt_t, qt_t, wk3):
                            ro = (h % 2) * 64
                            dt = h // 2
                            for qb, nkt in ((0, 8), (1, NT)):
                                psO = psa.tile([65, 512], F32, tag="acc",
                                                name="ps_O")
                                for g0 in range(0, nkt, 4):
                                    exps = []
                                    for pp in (g0 // 2, g0 // 2 + 1):
                                        psS2 = psp.tile([128, 2, 512], F32,
                                                        tag="tr", name="ps_S2")
                                        for i in range(2):
                                            kt = 2 * pp + i
                                            nc.tensor.matmul(
                                                psS2[:, i, :],
                                                kt_t[ro:ro + 64, ts(kt, 128)],
                                                qt_t[ro:ro + 64, ts(qb, 512)],
                                                start=True, stop=True)
                                        expP = wk3.tile([128, 2, 512], BF16,
                                                        tag="expP", name="expP",
                                                        bufs=4)
                                        nc.scalar.activation(expP[:], psS2[:],
                                                             AF.Exp, scale=0.125)
                                        if qb == 0 or pp >= 4:
                                            nc.vector.tensor_mul(expP[:], expP[:],
                                                                 mask_sb[pp][:])
                                        exps.append(expP)
                                    for i, kt in enumerate(range(g0, g0 + 4)):
                                        nc.tensor.matmul(
                                            psO[:], V_sb[kt][h // 8][:, h % 8, :],
                                            exps[i // 2][:, i % 2, :],
                                            start=(kt == 0), stop=(kt == nkt - 1))
                                rcp = wk3.tile([1, 512], F32, tag="rcp",
                                               name="rcp")
                                nc.vector.reciprocal(rcp[:], psO[64:65, :])
                                rep = wk3.tile([64, 512], F32, tag="rep",
                                               name="rep", bufs=2)
                                nc.gpsimd.partition_broadcast(rep[:], rcp[:],
                                                              channels=64)
                                nc.vector.tensor_mul(
                                    attnT[dt][ro:ro + 64, ts(qb, 512)],
                                    psO[0:64, :], rep[:])

                        with tc.tile_pool(name="wk3", bufs=4) as wk3:
                            for half in range(2):
                                v_block(half)
                                for dt in range(half * 4, (half + 1) * 4):
                                    kt_t = pa.tile([128, T], BF16, tag="KT",
                                                   name="KT", bufs=2)
                                    qt_t = pa.tile([128, TQ], BF16, tag="QT",
                                                   name="QT", bufs=2)
                                    qkv_chain(
                                        kt_t, lambda d, b: d[:, ts(b, 512)],
                                        C + dt * 128, T // 512,
                                        lambda cc, b: xsT[cc][:, ts(b, 512)])
                                    qkv_chain(
                                        qt_t, lambda d, b: d[:, ts(b, 512)],
                                        dt * 128, TQ // 512,
                                        lambda cc, b: xqsT[cc][:, ts(b, 512)])
                                    attention_head(2 * dt, kt_t, qt_t, wk3)
                                    attention_head(2 * dt + 1, kt_t, qt_t, wk3)

                # ---------- Phase 4: proj + residual (att pool freed) ----------
                with tc.tile_pool(name="px2", bufs=1) as px2:
                    x2_sb = [px2.tile([128, C], F32, tag=f"x2_{qt}",
                                      name=f"x2_{qt}") for qt in range(NQ)]
                    with tc.tile_pool(name="w4", bufs=3) as wp4, \
                         tc.tile_pool(name="wk4", bufs=2) as wk4:
                        ones_row = None
                        if has_bias:
                            ones_row = px2.tile([1, TQ], BF16, tag="ones_row",
                                                name="ones_row")
                            nc.vector.memset(ones_row[:], 1.0)
                        w_ts = {}
                        for cb in range(2):
                            for ht in range(NC8):
                                w_t = wp4.tile([128, 512], BF16, tag="wp_t",
                                               name="wp_t", bufs=18)
                                nc.sync.dma_start(w_t[:],
                                                  wpaug[ts(ht, 128), ts(cb, 512)])
                                w_ts[(cb, ht)] = w_t
                            if has_bias:
                                w_aug = wp4.tile([1, 512], BF16, tag="wp_aug",
                                                 name="wp_aug", bufs=2)
                                nc.sync.dma_start(w_aug[:],
                                                  wpaug[C:C + 1, ts(cb, 512)])
                                w_ts[(cb, "aug")] = w_aug
                        for qt in range(NQ):
                            for cb in range(2):
                                ps = psa.tile([128, 512], F32, tag="acc",
                                              name="ps_p")
                                for ht in range(NC8):
                                    nc.tensor.matmul(
                                        ps[:], attnT[ht][:, ts(qt, 128)],
                                        w_ts[(cb, ht)][:],
                                        start=(ht == 0),
                                        stop=(ht == NC8 - 1 and not has_bias))
                                if has_bias:
                                    nc.tensor.matmul(ps[:],
                                                     ones_row[:, ts(qt, 128)],
                                                     w_ts[(cb, "aug")][:],
                                                     start=False, stop=True)
                                xq_t = wk4.tile([128, 512], F32, tag="xq_t",
                                                name="xq_t")
                                nc.sync.dma_start(xq_t[:],
                                                  xq[ts(qt, 128), ts(cb, 512)])
                                nc.vector.tensor_add(x2_sb[qt][:, ts(cb, 512)],
                                                     ps[:], xq_t[:])

                    _mlp(nc, tc, sp, psa, psp, ident, x2_sb, w2aug, w3aug, out,
                         gelu_fn, has_bias)

    nc.compile()
    return nc


def _mlp(nc, tc, sp, psa, psp, ident, x2_sb, w2aug, w3aug, out, gelu_fn,
         has_bias):
    # ---------- Phase 5: LN2 + transpose; 6: fc1+gelu; 7: fc2+residual ------
    with tc.tile_pool(name="pgel", bufs=1) as pgel, \
         tc.tile_pool(name="w7", bufs=3) as wp7:
        geluT = [pgel.tile([128, TQ], BF16, tag=f"geluT{ft}", name=f"geluT{ft}")
                 for ft in range(NF)]
        ones_b16 = None
        if has_bias:
            ones_b16 = pgel.tile([1, TQ], BF16, tag="ones_b16", name="ones_b16")
            nc.vector.memset(ones_b16[:], 1.0)
        w3_ts = {}
        for cb in range(2):
            for ft in range(NF):
                w_t = wp7.tile([128, 512], BF16, tag="w3_t", name="w3_t",
                               bufs=NF + 2)
                nc.sync.dma_start(w_t[:], w3aug[ts(ft, 128), ts(cb, 512)])
                w3_ts[(cb, ft)] = w_t
            if has_bias:
                w_aug = wp7.tile([1, 512], BF16, tag="w3_aug", name="w3_aug")
                nc.sync.dma_start(w_aug[:], w3aug[4 * C:4 * C + 1, ts(cb, 512)])
                w3_ts[(cb, "aug")] = w_aug

        with tc.tile_pool(name="ph56", bufs=1) as p56, \
             tc.tile_pool(name="w6", bufs=3) as wp6, \
             tc.tile_pool(name="wk5", bufs=2) as wk5:
            xs2T = [p56.tile([128, TQ], BF16, tag=f"xs2T{cc}", name=f"xs2T{cc}")
                    for cc in range(NC8)]
            ones2 = None
            if has_bias:
                ones2 = p56.tile([1, TQ], BF16, tag="ones2", name="ones2")
                nc.vector.memset(ones2[:, :], 1.0)

            _ln_transpose(nc, sp, psp, ident, lambda qt: x2_sb[qt][:],
                          xs2T, NQ, wk5)

            # fc1 + gelu
            for ft in range(NF):
                w_t = wp6.tile([128, NC8, 128], BF16, tag="w2_t", name="w2_t",
                               bufs=3)
                nc.sync.dma_start(
                    w_t[:],
                    w2aug[0:C, ts(ft, 128)].rearrange("(cc p) n -> p cc n",
                                                      p=128))
                if has_bias:
                    w_aug = wp6.tile([1, 128], BF16, tag="w2_aug", name="w2_aug")
                    nc.sync.dma_start(w_aug[:], w2aug[C:C + 1, ts(ft, 128)])
                for tb in range(TQ // 512):
                    ps = psa.tile([128, 512], F32, tag="acc", name="ps_f1")
                    for cc in range(NC8):
                        nc.tensor.matmul(ps[:], w_t[:, cc, :],
                                         xs2T[cc][:, ts(tb, 512)],
                                         start=(cc == 0),
                                         stop=(cc == NC8 - 1 and not has_bias))
                    if has_bias:
                        nc.tensor.matmul(ps[:], w_aug[:],
                                         ones2[:, ts(tb, 512)],
                                         start=False, stop=True)
                    nc.scalar.activation(geluT[ft][:, ts(tb, 512)], ps[:], gelu_fn)

        # fc2 + residual
        with tc.tile_pool(name="wk7", bufs=2) as wk7:
            for cb in range(2):
                for qt in range(NQ):
                    ps = psa.tile([128, 512], F32, tag="acc", name="ps_f2")
                    for ft in range(NF):
                        nc.tensor.matmul(ps[:], geluT[ft][:, ts(qt, 128)],
                                         w3_ts[(cb, ft)][:],
                                         start=(ft == 0),
                                         stop=(ft == NF - 1 and not has_bias))
                    if has_bias:
                        nc.tensor.matmul(ps[:], ones_b16[:, ts(qt, 128)],
                                         w3_ts[(cb, "aug")][:],
                                         start=False, stop=True)
                    out_t = wk7.tile([128, 512], F32, tag="out_t", name="out_t")
                    nc.vector.tensor_add(out_t[:], ps[:],
                                         x2_sb[qt][:, ts(cb, 512)])
                    nc.sync.dma_start(out[ts(qt, 128), ts(cb, 512)], out_t[:])


def host_prep(inputs):
    """Build per-core input maps (all numpy, layout/weight-folding only)."""
    x = np.asarray(inputs["hidden_states"], np.float32)
    w_attn = np.asarray(inputs["w_attn"], np.float32)
    b_attn = np.asarray(inputs["b_attn"], np.float32)
    w_proj = np.asarray(inputs["w_proj"], np.float32)
    b_proj = np.asarray(inputs["b_proj"], np.float32)
    ln1_g = np.asarray(inputs["ln1_g"], np.float32)
    ln1_b = np.asarray(inputs["ln1_b"], np.float32)
    ln2_g = np.asarray(inputs["ln2_g"], np.float32)
    ln2_b = np.asarray(inputs["ln2_b"], np.float32)
    w_fc = np.asarray(inputs["w_fc"], np.float32)
    b_fc = np.asarray(inputs["b_fc"], np.float32)
    w_fc2 = np.asarray(inputs["w_fc2"], np.float32)
    b_fc2 = np.asarray(inputs["b_fc2"], np.float32)

    W1 = ln1_g[:, None] * w_attn
    bias1 = ln1_b @ w_attn + b_attn
    w1aug = np.concatenate([W1, bias1[None, :]], 0).astype(ml_dtypes.bfloat16)
    wpaug = np.concatenate([w_proj, b_proj[None, :]], 0).astype(ml_dtypes.bfloat16)
    W2 = ln2_g[:, None] * w_fc
    bias2 = ln2_b @ w_fc + b_fc
    w2aug = np.concatenate([W2, bias2[None, :]], 0).astype(ml_dtypes.bfloat16)
    w3aug = np.concatenate([w_fc2, b_fc2[None, :]], 0).astype(ml_dtypes.bfloat16)
    has_bias = bool(np.any(bias1) or np.any(bias2) or np.any(b_proj)
                    or np.any(b_fc2))

    in_maps = []
    slices = []
    karr = np.arange(T)
    for c in range(NCORES):
        b, j = c // 2, c % 2
        blockA = np.arange(j * 512, (j + 1) * 512)
        blockB = np.arange(1024 + j * 512, 1536 + j * 512)
        own = np.concatenate([blockA, blockB])
        xq_np = np.ascontiguousarray(x[b][own])
        maskc = np.empty((T, 512), np.float32)
        maskc[:1024] = (karr[:1024, None] <= blockA[None, :])
        maskc[1024:] = (karr[1024:, None] <= blockB[None, :])
        in_maps.append({
            "xb": np.ascontiguousarray(x[b]), "xq": xq_np,
            "maskc": maskc.astype(ml_dtypes.bfloat16),
            "w1aug": w1aug, "wpaug": wpaug, "w2aug": w2aug, "w3aug": w3aug,
        })
        slices.append((b, own))
    return in_maps, slices, has_bias


_NC_CACHE = {}


def kernel(**inputs):
    in_maps, slices, has_bias = host_prep(inputs)
    if has_bias not in _NC_CACHE:
        _NC_CACHE[has_bias] = build_program(has_bias=has_bias)
    nc = _NC_CACHE[has_bias]
    res = run_bass_kernel_spmd(nc, in_maps, list(range(NCORES)))
    out = np.empty((B, T, C), np.float32)
    for c, (b, own) in enumerate(slices):
        out[b, own] = res.results[c]["out"]
    return out



# revision 6
# speedup vs baseline: 1.0088x; 1.0088x over previous
"""Trainium2 Bass kernel for a GPT-2 transformer layer (B=4, T=2048, C=1024, H=16).

Sharding: 8 cores, one batch per core-pair; each core owns 1024 query tokens
(two 512-row blocks chosen so per-core causal attention work is balanced and
the SPMD program is uniform). No collectives: each core computes K/V for all
2048 tokens of its batch, attention + MLP for its own rows only.

Precision: QKV projections, V, and the attention-output projection run as
fp8e4m3 DoubleRow matmuls (2 contraction tiles per instruction = 2x tensor
throughput); QK^T, PV, and the MLP stay bf16 for accuracy. Scales: weights
are quantized at 16x (fp8 subnormal floor), so S^T sits at 256x (absorbed by
the exp scale), V is drained at 1/16, attnT is written at 32x (fp8 range),
and the proj drain descale is 1/512.

LayerNorms are folded into the matmuls (gamma into W, beta/bias via an
augmented ones-row matmul when biases are nonzero). Attention runs transposed
(S^T[k,q] tiles): softmax denominators come from an extra ones-column in the
PV stationary; per-head 1/den applies after PV (gpsimd partition-broadcast +
multiply). Causality lives in host-built multiplicative mask data so the
SPMD program is uniform across cores.
"""

import numpy as np
import ml_dtypes

import concourse.bass as bass
import concourse.mybir as mybir
import concourse.tile as tile
from concourse import bacc
from concourse.bass import ts
from concourse.bass_utils import run_bass_kernel_spmd
from concourse.masks import make_identity

B, T, C, H = 4, 2048, 1024, 16
D = C // H          # 64
TQ = T // 2         # own query tokens per core = 1024
NCORES = 8
EPS = 1e-5

F32 = mybir.dt.float32
BF16 = mybir.dt.bfloat16
F8 = mybir.dt.float8e4
DR = mybir.MatmulPerfMode.DoubleRow
AF = mybir.ActivationFunctionType

NT = T // 128        # 16 token tiles (all tokens)
NQ = TQ // 128       # 8 token tiles (own tokens)
NC8 = C // 128       # 8 c tiles
NCI = 4              # 4 c tile-pairs (DoubleRow)
NF = 4 * C // 128    # 32 fc hidden tiles

SW = 16.0            # fp8 weight scale
SA = 32.0            # attnT fp8 scale
EXP_SCALE = 0.125 / (SW * SW)
PROJ_DESCALE = 1.0 / (SA * SW)


def _ln_stats(nc, pool, x_t, ncols):
    """Per-partition mean/rsqrt stats of a [128, ncols] tile.
    Returns (mu, r): r = rsqrt(var+eps), both [128, 1] f32."""
    s = pool.tile([128, 1], F32, tag="ln_sum", name="ln_sum")
    ss = pool.tile([128, 1], F32, tag="ln_ssq", name="ln_ssq")
    trash = pool.tile([128, ncols], BF16, tag="ln_trash", name="ln_trash", bufs=1)
    nc.scalar.activation(trash[:], x_t[:], AF.Copy, accum_out=s[:])
    nc.scalar.activation(trash[:], x_t[:], AF.Square, accum_out=ss[:])
    mu = pool.tile([128, 1], F32, tag="ln_mu", name="ln_mu")
    nc.vector.tensor_scalar_mul(mu[:], s[:], 1.0 / ncols)
    ex2 = pool.tile([128, 1], F32, tag="ln_ex2", name="ln_ex2")
    nc.vector.tensor_scalar_mul(ex2[:], ss[:], 1.0 / ncols)
    var = pool.tile([128, 1], F32, tag="ln_var", name="ln_var")
    nc.vector.tensor_mul(var[:], mu[:], mu[:])
    nc.vector.tensor_sub(var[:], ex2[:], var[:])
    nc.vector.tensor_scalar_add(var[:], var[:], EPS)
    std = pool.tile([128, 1], F32, tag="ln_std", name="ln_std")
    nc.scalar.sqrt(std[:], var[:])
    r = pool.tile([128, 1], F32, tag="ln_r", name="ln_r")
    nc.vector.reciprocal(r[:], std[:])
    return mu, r


def _ln_transpose(nc, sp, psp, ident, src_tile, store, n_tiles, wk):
    """LayerNorm ((x-mu)*rsqrt) token-major [128, C] tiles into bf16 and
    PE-transpose; store(cc, tt0, pst4) writes each 4-tile PSUM drain
    (the store's destination dtype applies the final cast)."""
    for tt0 in range(0, n_tiles, 4):
        xs_ts = []
        for tt in range(tt0, tt0 + 4):
            x_t = src_tile(tt)
            mu, r = _ln_stats(nc, sp, x_t, C)
            xs_t = wk.tile([128, C], BF16, tag="xs_t", name="xs_t", bufs=5)
            nc.vector.tensor_scalar(xs_t[:], x_t[:], mu[:], r[:],
                                    mybir.AluOpType.subtract,
                                    mybir.AluOpType.mult)
            xs_ts.append(xs_t)
        for cc in range(NC8):
            pst4 = psp.tile([128, 4, 128], BF16, tag="tr", name="pst4")
            for i in range(4):
                nc.tensor.transpose(pst4[:, i, :],
                                    xs_ts[i][:, ts(cc, 128)], ident[:])
            store(cc, tt0, pst4)


def build_program(gelu_fn=None, loop_n=1, has_bias=False):
    nc = bacc.Bacc("TRN2", target_bir_lowering=False, debug=False)
    if gelu_fn is None:
        gelu_fn = AF.Gelu

    xb = nc.dram_tensor("xb", [T, C], F32, kind="ExternalInput")
    xq = nc.dram_tensor("xq", [TQ, C], F32, kind="ExternalInput")
    maskc = nc.dram_tensor("maskc", [T, 512], BF16, kind="ExternalInput")
    # fp8 DoubleRow weights: per 128-col block b (8 K then 8 Q interleaved as
    # [dt][0]=K,[1]=Q), flat [128, ci*two*128]; V/proj as [2, 128, ci*two*512]
    w1kq = nc.dram_tensor("w1kq", [16, 128, 8 * 128], F8, kind="ExternalInput")
    w1v = nc.dram_tensor("w1v", [2, 128, 8 * 512], F8, kind="ExternalInput")
    wp2 = nc.dram_tensor("wp2", [2, 128, 8 * 512], F8, kind="ExternalInput")
    w2aug = nc.dram_tensor("w2aug", [C + 1, 4 * C], BF16, kind="ExternalInput")
    w3aug = nc.dram_tensor("w3aug", [4 * C + 1, C], BF16, kind="ExternalInput")
    if has_bias:
        b1kq = nc.dram_tensor("b1kq", [16, 128], BF16, kind="ExternalInput")
        b1v = nc.dram_tensor("b1v", [2, 512], BF16, kind="ExternalInput")
        bp2 = nc.dram_tensor("bp2", [2, 512], BF16, kind="ExternalInput")
    out = nc.dram_tensor("out", [TQ, C], F32, kind="ExternalOutput")

    with tile.TileContext(nc) as tc:
        with (
            tc.tile_pool(name="glob", bufs=1) as pg,
            tc.tile_pool(name="stats", bufs=2) as sp,
            tc.tile_pool(name="psacc", bufs=4, space="PSUM") as psa,
            tc.tile_pool(name="pstr", bufs=2, space="PSUM") as psp,
        ):
            ident = pg.tile([128, 128], BF16, tag="ident", name="ident")
            make_identity(nc, ident[:])

            import contextlib
            loop_cm = tc.For_i(0, loop_n, 1) if loop_n > 1 else contextlib.nullcontext()
            with loop_cm, tc.tile_pool(name="p34", bufs=1) as p34:
                attnT = [p34.tile([128, 2, TQ], F8, tag=f"attnT{di}",
                                  name=f"attnT{di}") for di in range(NCI)]

                with tc.tile_pool(name="att", bufs=1) as pa:
                    V_sb = [[pa.tile([128, 8, 65], BF16, tag=f"V{tt}_{hb}",
                                     name=f"V{tt}_{hb}") for hb in range(2)]
                            for tt in range(NT)]
                    mask_sb = [pa.tile([128, 2, 512], BF16, tag=f"mask{pp}",
                                       name=f"mask{pp}") for pp in range(NT // 2)]
                    for pp in range(NT // 2):
                        nc.sync.dma_start(
                            mask_sb[pp][:],
                            maskc[pp * 256:(pp + 1) * 256, :].rearrange(
                                "(i p) q -> p i q", p=128))

                    # ---------- Phase 1: LN1 + transpose (xb and xq) ----------
                    with tc.tile_pool(name="ph12", bufs=1) as p12, \
                         tc.tile_pool(name="w12", bufs=3) as wp, \
                         tc.tile_pool(name="wk12", bufs=2) as wk:
                        xsT = [p12.tile([128, 2, T], F8, tag=f"xsT{ci}",
                                        name=f"xsT{ci}") for ci in range(NCI)]
                        xqsT = [p12.tile([128, 2, TQ], F8, tag=f"xqsT{ci}",
                                         name=f"xqsT{ci}") for ci in range(NCI)]
                        ones_t = None
                        if has_bias:
                            ones_t = p12.tile([1, T], BF16, tag="ones_t",
                                              name="ones_t")
                            nc.vector.memset(ones_t[:, :], 1.0)

                        def _load_xb(tt):
                            t = wk.tile([128, C], F32, tag="xb_t", name="xb_t")
                            nc.sync.dma_start(t[:], xb[ts(tt, 128), :])
                            return t

                        def _load_xq(tt):
                            t = wk.tile([128, C], F32, tag="xb_t", name="xq_t")
                            nc.sync.dma_start(t[:], xq[ts(tt, 128), :])
                            return t

                        def _store_xsT(cc, tt0, pst4):
                            nc.vector.tensor_copy(
                                xsT[cc // 2][:, cc % 2,
                                             tt0 * 128:(tt0 + 4) * 128], pst4[:])

                        def _store_xqsT(cc, tt0, pst4):
                            nc.vector.tensor_copy(
                                xqsT[cc // 2][:, cc % 2,
                                              tt0 * 128:(tt0 + 4) * 128], pst4[:])

                        _ln_transpose(nc, sp, psp, ident, _load_xb,
                                      _store_xsT, NT, wk)
                        _ln_transpose(nc, sp, psp, ident, _load_xq,
                                      _store_xqsT, NQ, wk)

                        # ---------- Phase 2+3: QKV + attention, interleaved ----
                        def qkv_chain(dst, blk_idx, n_blk, srcT):
                            """One K/Q column-block chain: 4 fp8 DoubleRow
                            matmuls (+ bf16 bias)."""
                            w_t = wp.tile([128, NCI, 2, 128], F8, tag="w1_t",
                                          name="w1_t", bufs=3)
                            nc.sync.dma_start(
                                w_t[:],
                                w1kq[blk_idx].rearrange(
                                    "p (ci two n) -> p ci two n", ci=4, two=2))
                            b_aug = None
                            if has_bias:
                                b_aug = wp.tile([1, 128], BF16, tag="w1_aug",
                                                name="w1_aug")
                                nc.sync.dma_start(b_aug[:],
                                                  b1kq[blk_idx:blk_idx + 1, :])
                            for blk in range(n_blk):
                                ps = psa.tile([128, 512], F32, tag="acc",
                                              name="ps_qkv")
                                for ci in range(NCI):
                                    nc.tensor.matmul(ps[:], w_t[:, ci, :, :],
                                                     srcT[ci][:, :, ts(blk, 512)],
                                                     start=(ci == 0),
                                                     stop=(ci == NCI - 1
                                                           and not has_bias),
                                                     perf_mode=DR)
                                if has_bias:
                                    nc.tensor.matmul(ps[:], b_aug[:],
                                                     ones_t[:, ts(blk, 512)],
                                                     start=False, stop=True)
                                nc.vector.tensor_copy(dst[:, ts(blk, 512)], ps[:])

                        # V for all heads (token-major), per hd-block
                        def v_block(hb):
                            w_t = wp.tile([128, NCI, 2, 512], F8, tag="w1v_t",
                                          name="w1v_t", bufs=2)
                            nc.sync.dma_start(
                                w_t[:],
                                w1v[hb].rearrange(
                                    "p (ci two n) -> p ci two n", ci=4, two=2))
                            b_aug = None
                            if has_bias:
                                b_aug = wp.tile([1, 512], BF16, tag="w1v_aug",
                                                name="w1v_aug", bufs=2)
                                nc.sync.dma_start(b_aug[:], b1v[hb:hb + 1, :])
                            for tt in range(NT):
                                ps = psa.tile([128, 512], F32, tag="acc",
                                              name="ps_v")
                                for ci in range(NCI):
                                    nc.tensor.matmul(ps[:],
                                                     xsT[ci][:, :, ts(tt, 128)],
                                                     w_t[:, ci, :, :],
                                                     start=(ci == 0),
                                                     stop=(ci == NCI - 1
                                                           and not has_bias),
                                                     perf_mode=DR)
                                if has_bias:
                                    nc.tensor.matmul(ps[:],
                                                     ones_t[:, ts(tt, 128)],
                                                     b_aug[:],
                                                     start=False, stop=True)
                                vt = V_sb[tt][hb]
                                nc.vector.tensor_scalar_mul(
                                    vt[:, :, 0:64],
                                    ps[:].rearrange("p (h d) -> p h d", h=8),
                                    1.0 / SW)
                                nc.vector.memset(vt[:, :, 64:65], 1.0)

                        def attention_head(h, kt_t, qt_t, wk3):
                            ro = (h % 2) * 64
                            di, dj = (h // 2) // 2, (h // 2) % 2
                            for qb, nkt in ((0, 8), (1, NT)):
                                psO = psa.tile([65, 512], F32, tag="acc",
                                                name="ps_O")
                                for g0 in range(0, nkt, 4):
                                    exps = []
                                    for pp in (g0 // 2, g0 // 2 + 1):
                                        psS2 = psp.tile([128, 2, 512], F32,
                                                        tag="tr", name="ps_S2")
                                        for i in range(2):
                                            kt = 2 * pp + i
                                            nc.tensor.matmul(
                                                psS2[:, i, :],
                                                kt_t[ro:ro + 64, ts(kt, 128)],
                                                qt_t[ro:ro + 64, ts(qb, 512)],
                                                start=True, stop=True)
                                        expP = wk3.tile([128, 2, 512], BF16,
                                                        tag="expP", name="expP",
                                                        bufs=4)
                                        nc.scalar.activation(expP[:], psS2[:],
                                                             AF.Exp,
                                                             scale=EXP_SCALE)
                                        if qb == 0 or pp >= 4:
                                            nc.vector.tensor_mul(expP[:], expP[:],
                                                                 mask_sb[pp][:])
                                        exps.append(expP)
                                    for i, kt in enumerate(range(g0, g0 + 4)):
                                        nc.tensor.matmul(
                                            psO[:], V_sb[kt][h // 8][:, h % 8, :],
                                            exps[i // 2][:, i % 2, :],
                                            start=(kt == 0), stop=(kt == nkt - 1))
                                rcp = wk3.tile([1, 512], F32, tag="rcp",
                                               name="rcp")
                                nc.vector.reciprocal(rcp[:], psO[64:65, :])
                                rcs = wk3.tile([1, 512], F32, tag="rcs",
                                               name="rcs")
                                nc.vector.tensor_scalar_mul(rcs[:], rcp[:], SA)
                                rep = wk3.tile([64, 512], F32, tag="rep",
                                               name="rep", bufs=2)
                                nc.gpsimd.partition_broadcast(rep[:], rcs[:],
                                                              channels=64)
                                nc.vector.tensor_mul(
                                    attnT[di][ro:ro + 64, dj, ts(qb, 512)],
                                    psO[0:64, :], rep[:])

                        with tc.tile_pool(name="wk3", bufs=4) as wk3:
                            for half in range(2):
                                v_block(half)
                                for dt in range(half * 4, (half + 1) * 4):
                                    kt_t = pa.tile([128, T], BF16, tag="KT",
                                                   name="KT", bufs=2)
                                    qt_t = pa.tile([128, TQ], BF16, tag="QT",
                                                   name="QT", bufs=2)
                                    qkv_chain(kt_t, 2 * dt, T // 512, xsT)
                                    qkv_chain(qt_t, 2 * dt + 1, TQ // 512, xqsT)
                                    attention_head(2 * dt, kt_t, qt_t, wk3)
                                    attention_head(2 * dt + 1, kt_t, qt_t, wk3)

                # ---------- Phase 4: proj + residual (att pool freed) ----------
                with tc.tile_pool(name="px2", bufs=1) as px2:
                    x2_sb = [px2.tile([128, C], F32, tag=f"x2_{qt}",
                                      name=f"x2_{qt}") for qt in range(NQ)]
                    with tc.tile_pool(name="w4", bufs=3) as wp4, \
                         tc.tile_pool(name="wk4", bufs=2) as wk4:
                        ones_row = None
                        if has_bias:
                            ones_row = px2.tile([1, TQ], BF16, tag="ones_row",
                                                name="ones_row")
                            nc.vector.memset(ones_row[:], 1.0)
                        w_ts = {}
                        for cb in range(2):
                            w_t = wp4.tile([128, NCI, 2, 512], F8, tag="wp_t",
                                           name="wp_t", bufs=2)
                            nc.sync.dma_start(
                                w_t[:],
                                wp2[cb].rearrange(
                                    "p (ci two n) -> p ci two n", ci=4, two=2))
                            w_ts[cb] = w_t
                            if has_bias:
                                b_aug = wp4.tile([1, 512], BF16, tag="wp_aug",
                                                 name="wp_aug", bufs=2)
                                nc.sync.dma_start(b_aug[:], bp2[cb:cb + 1, :])
                                w_ts[(cb, "aug")] = b_aug
                        for qt in range(NQ):
                            for cb in range(2):
                                ps = psa.tile([128, 512], F32, tag="acc",
                                              name="ps_p")
                                for ci in range(NCI):
                                    nc.tensor.matmul(
                                        ps[:], attnT[ci][:, :, ts(qt, 128)],
                                        w_ts[cb][:, ci, :, :],
                                        start=(ci == 0),
                                        stop=(ci == NCI - 1 and not has_bias),
                                        perf_mode=DR)
                                if has_bias:
                                    nc.tensor.matmul(ps[:],
                                                     ones_row[:, ts(qt, 128)],
                                                     w_ts[(cb, "aug")][:],
                                                     start=False, stop=True)
                                xq_t = wk4.tile([128, 512], F32, tag="xq_t",
                                                name="xq_t")
                                nc.sync.dma_start(xq_t[:],
                                                  xq[ts(qt, 128), ts(cb, 512)])
                                tmp = wk4.tile([128, 512], F32, tag="tmp",
                                               name="tmp")
                                nc.vector.tensor_scalar_mul(tmp[:], ps[:],
                                                            PROJ_DESCALE)
                                nc.vector.tensor_add(x2_sb[qt][:, ts(cb, 512)],
                                                     tmp[:], xq_t[:])

                    _mlp(nc, tc, sp, psa, psp, ident, x2_sb, w2aug, w3aug, out,
                         gelu_fn, has_bias)

    nc.compile()
    return nc


def _mlp(nc, tc, sp, psa, psp, ident, x2_sb, w2aug, w3aug, out, gelu_fn,
         has_bias):
    # ---------- Phase 5: LN2 + transpose; 6: fc1+gelu; 7: fc2+residual ------
    with tc.tile_pool(name="pgel", bufs=1) as pgel, \
         tc.tile_pool(name="w7", bufs=3) as wp7:
        geluT = [pgel.tile([128, TQ], BF16, tag=f"geluT{ft}", name=f"geluT{ft}")
                 for ft in range(NF)]
        ones_b16 = None
        if has_bias:
            ones_b16 = pgel.tile([1, TQ], BF16, tag="ones_b16", name="ones_b16")
            nc.vector.memset(ones_b16[:], 1.0)
        w3_ts = {}
        for cb in range(2):
            for ft in range(NF):
                w_t = wp7.tile([128, 512], BF16, tag="w3_t", name="w3_t",
                               bufs=NF + 2)
                nc.sync.dma_start(w_t[:], w3aug[ts(ft, 128), ts(cb, 512)])
                w3_ts[(cb, ft)] = w_t
            if has_bias:
                w_aug = wp7.tile([1, 512], BF16, tag="w3_aug", name="w3_aug")
                nc.sync.dma_start(w_aug[:], w3aug[4 * C:4 * C + 1, ts(cb, 512)])
                w3_ts[(cb, "aug")] = w_aug

        with tc.tile_pool(name="ph56", bufs=1) as p56, \
             tc.tile_pool(name="w6", bufs=3) as wp6, \
             tc.tile_pool(name="wk5", bufs=2) as wk5:
            xs2T = [p56.tile([128, TQ], BF16, tag=f"xs2T{cc}", name=f"xs2T{cc}")
                    for cc in range(NC8)]
            ones2 = None
            if has_bias:
                ones2 = p56.tile([1, TQ], BF16, tag="ones2", name="ones2")
                nc.vector.memset(ones2[:, :], 1.0)

            def _store_xs2T(cc, tt0, pst4):
                nc.vector.tensor_copy(
                    xs2T[cc][:, tt0 * 128:(tt0 + 4) * 128], pst4[:])

            _ln_transpose(nc, sp, psp, ident, lambda qt: x2_sb[qt][:],
                          _store_xs2T, NQ, wk5)

            # fc1 + gelu
            for ft in range(NF):
                w_t = wp6.tile([128, NC8, 128], BF16, tag="w2_t", name="w2_t",
                               bufs=3)
                nc.sync.dma_start(
                    w_t[:],
                    w2aug[0:C, ts(ft, 128)].rearrange("(cc p) n -> p cc n",
                                                      p=128))
                if has_bias:
                    w_aug = wp6.tile([1, 128], BF16, tag="w2_aug", name="w2_aug")
                    nc.sync.dma_start(w_aug[:], w2aug[C:C + 1, ts(ft, 128)])
                for tb in range(TQ // 512):
                    ps = psa.tile([128, 512], F32, tag="acc", name="ps_f1")
                    for cc in range(NC8):
                        nc.tensor.matmul(ps[:], w_t[:, cc, :],
                                         xs2T[cc][:, ts(tb, 512)],
                                         start=(cc == 0),
                                         stop=(cc == NC8 - 1 and not has_bias))
                    if has_bias:
                        nc.tensor.matmul(ps[:], w_aug[:],
                                         ones2[:, ts(tb, 512)],
                                         start=False, stop=True)
                    nc.scalar.activation(geluT[ft][:, ts(tb, 512)], ps[:], gelu_fn)

        # fc2 + residual
        with tc.tile_pool(name="wk7", bufs=2) as wk7:
            for cb in range(2):
                for qt in range(NQ):
                    ps = psa.tile([128, 512], F32, tag="acc", name="ps_f2")
                    for ft in range(NF):
                        nc.tensor.matmul(ps[:], geluT[ft][:, ts(qt, 128)],
                                         w3_ts[(cb, ft)][:],
                                         start=(ft == 0),
                                         stop=(ft == NF - 1 and not has_bias))
                    if has_bias:
                        nc.tensor.matmul(ps[:], ones_b16[:, ts(qt, 128)],
                                         w3_ts[(cb, "aug")][:],
                                         start=False, stop=True)
                    out_t = wk7.tile([128, 512], F32, tag="out_t", name="out_t")
                    nc.vector.tensor_add(out_t[:], ps[:],
                                         x2_sb[qt][:, ts(cb, 512)])
                    nc.sync.dma_start(out[ts(qt, 128), ts(cb, 512)], out_t[:])


def _dr_pack(W):
    """[C, N] -> flat [128, (C/256)*2*N] DoubleRow stationary layout:
    [p, ci, j, n] = W[(2ci+j)*128 + p, n]."""
    Cr, N = W.shape
    nci = Cr // 256
    Wr = W.reshape(nci, 2, 128, N).transpose(2, 0, 1, 3)   # [128, ci, 2, N]
    return np.ascontiguousarray(Wr.reshape(128, nci * 2 * N))


def host_prep(inputs):
    """Build per-core input maps (all numpy, layout/weight-folding only)."""
    x = np.asarray(inputs["hidden_states"], np.float32)
    w_attn = np.asarray(inputs["w_attn"], np.float32)
    b_attn = np.asarray(inputs["b_attn"], np.float32)
    w_proj = np.asarray(inputs["w_proj"], np.float32)
    b_proj = np.asarray(inputs["b_proj"], np.float32)
    ln1_g = np.asarray(inputs["ln1_g"], np.float32)
    ln1_b = np.asarray(inputs["ln1_b"], np.float32)
    ln2_g = np.asarray(inputs["ln2_g"], np.float32)
    ln2_b = np.asarray(inputs["ln2_b"], np.float32)
    w_fc = np.asarray(inputs["w_fc"], np.float32)
    b_fc = np.asarray(inputs["b_fc"], np.float32)
    w_fc2 = np.asarray(inputs["w_fc2"], np.float32)
    b_fc2 = np.asarray(inputs["b_fc2"], np.float32)

    F8NP = ml_dtypes.float8_e4m3

    W1 = ln1_g[:, None] * w_attn * SW
    bias1 = ln1_b @ w_attn + b_attn
    # K/Q 128-col blocks, interleaved [dt][K,Q]
    kq_blocks = []
    b_blocks = []
    for dt in range(NC8):
        kq_blocks.append(_dr_pack(W1[:, C + dt * 128:C + (dt + 1) * 128]))
        kq_blocks.append(_dr_pack(W1[:, dt * 128:(dt + 1) * 128]))
        b_blocks.append(bias1[C + dt * 128:C + (dt + 1) * 128] * SW)
        b_blocks.append(bias1[dt * 128:(dt + 1) * 128] * SW)
    w1kq = np.stack(kq_blocks).astype(F8NP)
    b1kq = np.stack(b_blocks).astype(ml_dtypes.bfloat16)
    w1v = np.stack([_dr_pack(W1[:, 2 * C + hb * 512:2 * C + (hb + 1) * 512])
                    for hb in range(2)]).astype(F8NP)
    b1v = np.stack([bias1[2 * C + hb * 512:2 * C + (hb + 1) * 512] * SW
                    for hb in range(2)]).astype(ml_dtypes.bfloat16)
    wp2 = np.stack([_dr_pack(w_proj[:, cb * 512:(cb + 1) * 512] * SW)
                    for cb in range(2)]).astype(F8NP)
    bp2 = np.stack([b_proj[cb * 512:(cb + 1) * 512] * (SA * SW)
                    for cb in range(2)]).astype(ml_dtypes.bfloat16)

    W2 = ln2_g[:, None] * w_fc
    bias2 = ln2_b @ w_fc + b_fc
    w2aug = np.concatenate([W2, bias2[None, :]], 0).astype(ml_dtypes.bfloat16)
    w3aug = np.concatenate([w_fc2, b_fc2[None, :]], 0).astype(ml_dtypes.bfloat16)
    has_bias = bool(np.any(bias1) or np.any(bias2) or np.any(b_proj)
                    or np.any(b_fc2))

    in_maps = []
    slices = []
    karr = np.arange(T)
    for c in range(NCORES):
        b, j = c // 2, c % 2
        blockA = np.arange(j * 512, (j + 1) * 512)
        blockB = np.arange(1024 + j * 512, 1536 + j * 512)
        own = np.concatenate([blockA, blockB])
        xq_np = np.ascontiguousarray(x[b][own])
        maskc = np.empty((T, 512), np.float32)
        maskc[:1024] = (karr[:1024, None] <= blockA[None, :])
        maskc[1024:] = (karr[1024:, None] <= blockB[None, :])
        m = {
            "xb": np.ascontiguousarray(x[b]), "xq": xq_np,
            "maskc": maskc.astype(ml_dtypes.bfloat16),
            "w1kq": w1kq, "w1v": w1v, "wp2": wp2,
            "w2aug": w2aug, "w3aug": w3aug,
        }
        if has_bias:
            m.update({"b1kq": b1kq, "b1v": b1v, "bp2": bp2})
        in_maps.append(m)
        slices.append((b, own))
    return in_maps, slices, has_bias


_NC_CACHE = {}


def kernel(**inputs):
    in_maps, slices, has_bias = host_prep(inputs)
    if has_bias not in _NC_CACHE:
        _NC_CACHE[has_bias] = build_program(has_bias=has_bias)
    nc = _NC_CACHE[has_bias]
    res = run_bass_kernel_spmd(nc, in_maps, list(range(NCORES)))
    out = np.empty((B, T, C), np.float32)
    for c, (b, own) in enumerate(slices):
        out[b, own] = res.results[c]["out"]
    return out


# revision 14
# speedup vs baseline: 1.1023x; 1.0927x over previous
"""Trainium2 Bass kernel for a GPT-2 transformer layer (B=4, T=2048, C=1024, H=16).

Sharding: 8 cores, one batch per core-pair; each core owns 1024 query tokens
(two 512-row blocks chosen so per-core causal attention work is balanced and
the SPMD program is uniform). No collectives: each core computes K/V for all
2048 tokens of its batch, attention + MLP for its own rows only.

Precision: QKV projections, V, and the attention-output projection run as
fp8e4m3 DoubleRow matmuls (2 contraction tiles per instruction = 2x tensor
throughput); QK^T, PV, and the MLP stay bf16 for accuracy. Scales: weights
are quantized at 16x (fp8 subnormal floor), so S^T sits at 256x (absorbed by
the exp scale), V is drained at 1/16, attnT is written at 32x (fp8 range),
and the proj drain descale is 1/512.

LayerNorms are folded into the matmuls (gamma into W, beta/bias via an
augmented ones-row matmul when biases are nonzero). Attention runs transposed
(S^T[k,q] tiles): softmax denominators come from an extra ones-column in the
PV stationary; per-head 1/den applies after PV (gpsimd partition-broadcast +
multiply). Causality lives in host-built multiplicative mask data so the
SPMD program is uniform across cores.
"""

import numpy as np
import ml_dtypes

import concourse.bass as bass
import concourse.mybir as mybir
import concourse.tile as tile
from concourse import bacc
from concourse.bass import ts
from concourse.bass_utils import run_bass_kernel_spmd
from concourse.masks import make_identity

B, T, C, H = 4, 2048, 1024, 16
D = C // H          # 64
TQ = T // 2         # own query tokens per core = 1024
NCORES = 8
EPS = 1e-5

F32 = mybir.dt.float32
BF16 = mybir.dt.bfloat16
F8 = mybir.dt.float8e4
DR = mybir.MatmulPerfMode.DoubleRow
AF = mybir.ActivationFunctionType

NT = T // 128        # 16 token tiles (all tokens)
NQ = TQ // 128       # 8 token tiles (own tokens)
NC8 = C // 128       # 8 c tiles
NCI = 4              # 4 c tile-pairs (DoubleRow)
NF = 4 * C // 128    # 32 fc hidden tiles

SW = 16.0            # fp8 weight scale
SA = 32.0            # attnT fp8 scale
EXP_SCALE = 0.125 / (SW * SW)
PROJ_DESCALE = 1.0 / (SA * SW)


def _ln_transpose(nc, sp, psp, ident, src_tile, store, n_tiles, wk):
    """LayerNorm ((x-mu)*rsqrt) token-major [128, C] tiles into bf16 and
    PE-transpose; store(cc, tt0, pst4) writes each 4-tile PSUM drain
    (the store's destination dtype applies the final cast).
    Stats are batched per 4-tile group: the accumulator passes pipeline on
    the scalar engine, then the [128,4] mean/var/rsqrt chain runs once."""
    for tt0 in range(0, n_tiles, 4):
        x_ts = [src_tile(tt) for tt in range(tt0, tt0 + 4)]
        s4 = sp.tile([128, 4], F32, tag="ln_s4", name="ln_s4")
        ss4 = sp.tile([128, 4], F32, tag="ln_ss4", name="ln_ss4")
        trash = sp.tile([128, C], BF16, tag="ln_trash", name="ln_trash", bufs=2)
        for i in range(4):
            nc.scalar.activation(trash[:], x_ts[i][:], AF.Copy,
                                 accum_out=s4[:, i:i + 1])
            nc.scalar.activation(trash[:], x_ts[i][:], AF.Square,
                                 accum_out=ss4[:, i:i + 1])
        mu4 = sp.tile([128, 4], F32, tag="ln_mu4", name="ln_mu4")
        nc.vector.tensor_scalar_mul(mu4[:], s4[:], 1.0 / C)
        var4 = sp.tile([128, 4], F32, tag="ln_var4", name="ln_var4")
        nc.vector.tensor_mul(var4[:], mu4[:], mu4[:])
        ex24 = sp.tile([128, 4], F32, tag="ln_ex24", name="ln_ex24")
        nc.vector.tensor_scalar_mul(ex24[:], ss4[:], 1.0 / C)
        nc.vector.tensor_sub(var4[:], ex24[:], var4[:])
        nc.vector.tensor_scalar_add(var4[:], var4[:], EPS)
        std4 = sp.tile([128, 4], F32, tag="ln_std4", name="ln_std4")
        nc.scalar.sqrt(std4[:], var4[:])
        r4 = sp.tile([128, 4], F32, tag="ln_r4", name="ln_r4")
        nc.vector.reciprocal(r4[:], std4[:])
        xs_ts = []
        for i in range(4):
            xs_t = wk.tile([128, C], BF16, tag="xs_t", name="xs_t", bufs=5)
            nc.vector.tensor_scalar(xs_t[:], x_ts[i][:], mu4[:, i:i + 1],
                                    r4[:, i:i + 1],
                                    mybir.AluOpType.subtract,
                                    mybir.AluOpType.mult)
            xs_ts.append(xs_t)
        for cc in range(NC8):
            pst4 = psp.tile([128, 4, 128], BF16, tag="tr", name="pst4")
            for i in range(4):
                nc.tensor.transpose(pst4[:, i, :],
                                    xs_ts[i][:, ts(cc, 128)], ident[:])
            store(cc, tt0, pst4)


def build_program(gelu_fn=None, loop_n=1, has_bias=False):
    nc = bacc.Bacc("TRN2", target_bir_lowering=False, debug=False)
    if gelu_fn is None:
        gelu_fn = AF.Gelu

    xb = nc.dram_tensor("xb", [T, C], F32, kind="ExternalInput")
    xq = nc.dram_tensor("xq", [TQ, C], F32, kind="ExternalInput")
    maskc = nc.dram_tensor("maskc", [T, 512], BF16, kind="ExternalInput")
    # fp8 DoubleRow weights: per 128-col block b (8 K then 8 Q interleaved as
    # [dt][0]=K,[1]=Q), flat [128, ci*two*128]; V/proj as [2, 128, ci*two*512]
    w1kq = nc.dram_tensor("w1kq", [16, 128, 8 * 128], F8, kind="ExternalInput")
    w1v = nc.dram_tensor("w1v", [2, 128, 8 * 512], F8, kind="ExternalInput")
    wp2 = nc.dram_tensor("wp2", [2, 128, 8 * 512], F8, kind="ExternalInput")
    w2aug = nc.dram_tensor("w2aug", [C + 1, 4 * C], BF16, kind="ExternalInput")
    w3aug = nc.dram_tensor("w3aug", [4 * C + 1, C], BF16, kind="ExternalInput")
    if has_bias:
        b1kq = nc.dram_tensor("b1kq", [16, 128], BF16, kind="ExternalInput")
        b1v = nc.dram_tensor("b1v", [2, 512], BF16, kind="ExternalInput")
        bp2 = nc.dram_tensor("bp2", [2, 512], BF16, kind="ExternalInput")
    out = nc.dram_tensor("out", [TQ, C], F32, kind="ExternalOutput")

    with tile.TileContext(nc) as tc:
        with (
            tc.tile_pool(name="glob", bufs=1) as pg,
            tc.tile_pool(name="stats", bufs=2) as sp,
            tc.tile_pool(name="psacc", bufs=2, space="PSUM") as psa,
            tc.tile_pool(name="pstr", bufs=2, space="PSUM") as psp,
            tc.tile_pool(name="psout", bufs=2, space="PSUM") as pso,
        ):
            ident = pg.tile([128, 128], BF16, tag="ident", name="ident")
            make_identity(nc, ident[:])

            import contextlib
            loop_cm = tc.For_i(0, loop_n, 1) if loop_n > 1 else contextlib.nullcontext()
            with loop_cm, tc.tile_pool(name="p34", bufs=1) as p34:
                attnT = [p34.tile([128, 2, TQ], F8, tag=f"attnT{di}",
                                  name=f"attnT{di}") for di in range(NCI)]

                with tc.tile_pool(name="att", bufs=1) as pa:
                    V_sb = [[pa.tile([128, 8, 65], BF16, tag=f"V{tt}_{hb}",
                                     name=f"V{tt}_{hb}") for hb in range(2)]
                            for tt in range(NT)]
                    mask_sb = [pa.tile([128, 2, 512], BF16, tag=f"mask{pp}",
                                       name=f"mask{pp}") for pp in range(NT // 2)]
                    for pp in range(NT // 2):
                        nc.sync.dma_start(
                            mask_sb[pp][:],
                            maskc[pp * 256:(pp + 1) * 256, :].rearrange(
                                "(i p) q -> p i q", p=128))

                    # ---------- Phase 1: LN1 + transpose (xb and xq) ----------
                    with tc.tile_pool(name="ph12", bufs=1) as p12, \
                         tc.tile_pool(name="w12", bufs=3) as wp, \
                         tc.tile_pool(name="wk12", bufs=2) as wk:
                        xsT = [p12.tile([128, 2, T], F8, tag=f"xsT{ci}",
                                        name=f"xsT{ci}") for ci in range(NCI)]
                        xqsT = [p12.tile([128, 2, TQ], F8, tag=f"xqsT{ci}",
                                         name=f"xqsT{ci}") for ci in range(NCI)]
                        ones_t = None
                        if has_bias:
                            ones_t = p12.tile([1, T], BF16, tag="ones_t",
                                              name="ones_t")
                            nc.vector.memset(ones_t[:, :], 1.0)

                        def _load_xb(tt):
                            t = wk.tile([128, C], F32, tag="xb_t", name="xb_t",
                                        bufs=6)
                            nc.sync.dma_start(t[:], xb[ts(tt, 128), :])
                            return t

                        def _load_xq(tt):
                            t = wk.tile([128, C], F32, tag="xb_t", name="xq_t",
                                        bufs=6)
                            nc.sync.dma_start(t[:], xq[ts(tt, 128), :])
                            return t

                        def _store_xsT(cc, tt0, pst4):
                            nc.vector.tensor_copy(
                                xsT[cc // 2][:, cc % 2,
                                             tt0 * 128:(tt0 + 4) * 128], pst4[:])

                        def _store_xqsT(cc, tt0, pst4):
                            nc.vector.tensor_copy(
                                xqsT[cc // 2][:, cc % 2,
                                              tt0 * 128:(tt0 + 4) * 128], pst4[:])

                        _ln_transpose(nc, sp, psp, ident, _load_xb,
                                      _store_xsT, NT, wk)
                        _ln_transpose(nc, sp, psp, ident, _load_xq,
                                      _store_xqsT, NQ, wk)

                        # ---------- Phase 2+3: QKV + attention, interleaved ----
                        def qkv_chain(dst, blk_idx, n_blk, srcT):
                            """One K/Q column-block chain: 4 fp8 DoubleRow
                            matmuls (+ bf16 bias)."""
                            w_t = wp.tile([128, NCI, 2, 128], F8, tag="w1_t",
                                          name="w1_t", bufs=3)
                            nc.sync.dma_start(
                                w_t[:],
                                w1kq[blk_idx].rearrange(
                                    "p (ci two n) -> p ci two n", ci=4, two=2))
                            b_aug = None
                            if has_bias:
                                b_aug = wp.tile([1, 128], BF16, tag="w1_aug",
                                                name="w1_aug")
                                nc.sync.dma_start(b_aug[:],
                                                  b1kq[blk_idx:blk_idx + 1, :])
                            for blk in range(n_blk):
                                ps = psa.tile([128, 512], F32, tag="acc",
                                              name="ps_qkv")
                                for ci in range(NCI):
                                    nc.tensor.matmul(ps[:], w_t[:, ci, :, :],
                                                     srcT[ci][:, :, ts(blk, 512)],
                                                     start=(ci == 0),
                                                     stop=(ci == NCI - 1
                                                           and not has_bias),
                                                     perf_mode=DR)
                                if has_bias:
                                    nc.tensor.matmul(ps[:], b_aug[:],
                                                     ones_t[:, ts(blk, 512)],
                                                     start=False, stop=True)
                                nc.vector.tensor_copy(dst[:, ts(blk, 512)], ps[:])

                        # V for all heads (token-major), per hd-block
                        def v_block(hb):
                            w_t = wp.tile([128, NCI, 2, 512], F8, tag="w1v_t",
                                          name="w1v_t", bufs=2)
                            nc.sync.dma_start(
                                w_t[:],
                                w1v[hb].rearrange(
                                    "p (ci two n) -> p ci two n", ci=4, two=2))
                            b_aug = None
                            if has_bias:
                                b_aug = wp.tile([1, 512], BF16, tag="w1v_aug",
                                                name="w1v_aug", bufs=2)
                                nc.sync.dma_start(b_aug[:], b1v[hb:hb + 1, :])
                            for tt in range(NT):
                                ps = psa.tile([128, 512], F32, tag="acc",
                                              name="ps_v")
                                for ci in range(NCI):
                                    nc.tensor.matmul(ps[:],
                                                     xsT[ci][:, :, ts(tt, 128)],
                                                     w_t[:, ci, :, :],
                                                     start=(ci == 0),
                                                     stop=(ci == NCI - 1
                                                           and not has_bias),
                                                     perf_mode=DR)
                                if has_bias:
                                    nc.tensor.matmul(ps[:],
                                                     ones_t[:, ts(tt, 128)],
                                                     b_aug[:],
                                                     start=False, stop=True)
                                vt = V_sb[tt][hb]
                                nc.vector.tensor_scalar_mul(
                                    vt[:, :, 0:64],
                                    ps[:].rearrange("p (h d) -> p h d", h=8),
                                    1.0 / SW)
                                nc.vector.memset(vt[:, :, 64:65], 1.0)

                        def attention_head(h, kt_t, qt_t, wk3):
                            """Both q-blocks of one head. PV runs one 4-ktile
                            group behind QK/exp so the PE never waits on the
                            scalar exp; softmax denominators for both q-blocks
                            batch into a single reciprocal."""
                            ro = (h % 2) * 64
                            di, dj = (h // 2) // 2, (h // 2) % 2
                            # rows at partitions 0/32 (offset granularity);
                            # fill so the batched reciprocal reads no
                            # uninitialized partitions
                            den2 = wk3.tile([33, 512], F32, tag="den2",
                                            name="den2", bufs=2)
                            nc.vector.memset(den2[:], 1.0)
                            psOs = {}
                            for qb, nkt in ((0, 8), (1, NT)):
                                psO = pso.tile([65, 512], F32, tag="psO",
                                               name="ps_O")
                                psOs[qb] = psO

                                def emit_pv(g0p, exps_p):
                                    for i, kt in enumerate(range(g0p, g0p + 4)):
                                        nc.tensor.matmul(
                                            psO[:], V_sb[kt][h // 8][:, h % 8, :],
                                            exps_p[i // 2][:, i % 2, :],
                                            start=(kt == 0), stop=(kt == nkt - 1))

                                pending = None
                                for g0 in range(0, nkt, 4):
                                    exps = []
                                    for pp in (g0 // 2, g0 // 2 + 1):
                                        psS2 = psp.tile([128, 2, 512], F32,
                                                        tag="tr", name="ps_S2")
                                        for i in range(2):
                                            kt = 2 * pp + i
                                            nc.tensor.matmul(
                                                psS2[:, i, :],
                                                kt_t[ro:ro + 64, ts(kt, 128)],
                                                qt_t[ro:ro + 64, ts(qb, 512)],
                                                start=True, stop=True)
                                        expP = wk3.tile([128, 2, 512], BF16,
                                                        tag="expP", name="expP",
                                                        bufs=6)
                                        nc.scalar.activation(expP[:], psS2[:],
                                                             AF.Exp,
                                                             scale=EXP_SCALE)
                                        if qb == 0 or pp >= 4:
                                            nc.vector.tensor_mul(expP[:], expP[:],
                                                                 mask_sb[pp][:])
                                        exps.append(expP)
                                    if pending is not None:
                                        emit_pv(*pending)
                                    pending = (g0, exps)
                                emit_pv(*pending)
                                nc.vector.tensor_copy(
                                    den2[32 * qb:32 * qb + 1, :],
                                    psO[64:65, :])
                            rep2 = wk3.tile([33, 512], F32, tag="rep2",
                                            name="rep2", bufs=2)
                            nc.vector.reciprocal(rep2[:], den2[:])
                            nc.vector.tensor_scalar_mul(rep2[:], rep2[:], SA)
                            # partition_broadcast reads partition 0 only —
                            # stage row 32 through a partition-0 tile
                            rep_b = wk3.tile([1, 512], F32, tag="rep_b",
                                             name="rep_b", bufs=2)
                            nc.vector.tensor_copy(rep_b[:], rep2[32:33, :])
                            for qb in (0, 1):
                                rep = wk3.tile([64, 512], F32, tag="rep",
                                               name="rep", bufs=2)
                                nc.gpsimd.partition_broadcast(
                                    rep[:], rep2[0:1, :] if qb == 0 else rep_b[:],
                                    channels=64)
                                nc.vector.tensor_mul(
                                    attnT[di][ro:ro + 64, dj, ts(qb, 512)],
                                    psOs[qb][0:64, :], rep[:])

                        with tc.tile_pool(name="wk3", bufs=4) as wk3:
                            for half in range(2):
                                v_block(half)
                                for dt in range(half * 4, (half + 1) * 4):
                                    kt_t = pa.tile([128, T], BF16, tag="KT",
                                                   name="KT", bufs=2)
                                    qt_t = pa.tile([128, TQ], BF16, tag="QT",
                                                   name="QT", bufs=2)
                                    qkv_chain(kt_t, 2 * dt, T // 512, xsT)
                                    qkv_chain(qt_t, 2 * dt + 1, TQ // 512, xqsT)
                                    attention_head(2 * dt, kt_t, qt_t, wk3)
                                    attention_head(2 * dt + 1, kt_t, qt_t, wk3)

                # ---------- Phase 4: proj + residual (att pool freed) ----------
                with tc.tile_pool(name="px2", bufs=1) as px2:
                    x2_sb = [px2.tile([128, C], F32, tag=f"x2_{qt}",
                                      name=f"x2_{qt}") for qt in range(NQ)]
                    with tc.tile_pool(name="w4", bufs=3) as wp4, \
                         tc.tile_pool(name="wk4", bufs=2) as wk4:
                        ones_row = None
                        if has_bias:
                            ones_row = px2.tile([1, TQ], BF16, tag="ones_row",
                                                name="ones_row")
                            nc.vector.memset(ones_row[:], 1.0)
                        w_ts = {}
                        for cb in range(2):
                            w_t = wp4.tile([128, NCI, 2, 512], F8, tag="wp_t",
                                           name="wp_t", bufs=2)
                            nc.sync.dma_start(
                                w_t[:],
                                wp2[cb].rearrange(
                                    "p (ci two n) -> p ci two n", ci=4, two=2))
                            w_ts[cb] = w_t
                            if has_bias:
                                b_aug = wp4.tile([1, 512], BF16, tag="wp_aug",
                                                 name="wp_aug", bufs=2)
                                nc.sync.dma_start(b_aug[:], bp2[cb:cb + 1, :])
                                w_ts[(cb, "aug")] = b_aug
                        for qt in range(NQ):
                            for cb in range(2):
                                ps = psa.tile([128, 512], F32, tag="acc",
                                              name="ps_p")
                                for ci in range(NCI):
                                    nc.tensor.matmul(
                                        ps[:], attnT[ci][:, :, ts(qt, 128)],
                                        w_ts[cb][:, ci, :, :],
                                        start=(ci == 0),
                                        stop=(ci == NCI - 1 and not has_bias),
                                        perf_mode=DR)
                                if has_bias:
                                    nc.tensor.matmul(ps[:],
                                                     ones_row[:, ts(qt, 128)],
                                                     w_ts[(cb, "aug")][:],
                                                     start=False, stop=True)
                                xq_t = wk4.tile([128, 512], F32, tag="xq_t",
                                                name="xq_t")
                                nc.sync.dma_start(xq_t[:],
                                                  xq[ts(qt, 128), ts(cb, 512)])
                                tmp = wk4.tile([128, 512], F32, tag="tmp",
                                               name="tmp")
                                nc.vector.tensor_scalar_mul(tmp[:], ps[:],
                                                            PROJ_DESCALE)
                                nc.vector.tensor_add(x2_sb[qt][:, ts(cb, 512)],
                                                     tmp[:], xq_t[:])

                    _mlp(nc, tc, sp, psa, psp, ident, x2_sb, w2aug, w3aug, out,
                         gelu_fn, has_bias)

    nc.compile()
    return nc


def _mlp(nc, tc, sp, psa, psp, ident, x2_sb, w2aug, w3aug, out, gelu_fn,
         has_bias):
    # ---------- Phase 5: LN2 + transpose; 6: fc1+gelu; 7: fc2+residual ------
    with tc.tile_pool(name="pgel", bufs=1) as pgel, \
         tc.tile_pool(name="w7", bufs=3) as wp7:
        geluT = [pgel.tile([128, TQ], BF16, tag=f"geluT{ft}", name=f"geluT{ft}")
                 for ft in range(NF)]
        ones_b16 = None
        if has_bias:
            ones_b16 = pgel.tile([1, TQ], BF16, tag="ones_b16", name="ones_b16")
            nc.vector.memset(ones_b16[:], 1.0)
        w3_ts = {}
        for cb in range(2):
            for ft in range(NF):
                w_t = wp7.tile([128, 512], BF16, tag="w3_t", name="w3_t",
                               bufs=NF + 2)
                nc.sync.dma_start(w_t[:], w3aug[ts(ft, 128), ts(cb, 512)])
                w3_ts[(cb, ft)] = w_t
            if has_bias:
                w_aug = wp7.tile([1, 512], BF16, tag="w3_aug", name="w3_aug")
                nc.sync.dma_start(w_aug[:], w3aug[4 * C:4 * C + 1, ts(cb, 512)])
                w3_ts[(cb, "aug")] = w_aug

        with tc.tile_pool(name="ph56", bufs=1) as p56, \
             tc.tile_pool(name="w6", bufs=3) as wp6, \
             tc.tile_pool(name="wk5", bufs=2) as wk5:
            xs2T = [p56.tile([128, TQ], BF16, tag=f"xs2T{cc}", name=f"xs2T{cc}")
                    for cc in range(NC8)]
            ones2 = None
            if has_bias:
                ones2 = p56.tile([1, TQ], BF16, tag="ones2", name="ones2")
                nc.vector.memset(ones2[:, :], 1.0)

            def _store_xs2T(cc, tt0, pst4):
                nc.vector.tensor_copy(
                    xs2T[cc][:, tt0 * 128:(tt0 + 4) * 128], pst4[:])

            _ln_transpose(nc, sp, psp, ident, lambda qt: x2_sb[qt][:],
                          _store_xs2T, NQ, wk5)

            # fc1 + gelu
            for ft in range(NF):
                w_t = wp6.tile([128, NC8, 128], BF16, tag="w2_t", name="w2_t",
                               bufs=3)
                nc.sync.dma_start(
                    w_t[:],
                    w2aug[0:C, ts(ft, 128)].rearrange("(cc p) n -> p cc n",
                                                      p=128))
                if has_bias:
                    w_aug = wp6.tile([1, 128], BF16, tag="w2_aug", name="w2_aug")
                    nc.sync.dma_start(w_aug[:], w2aug[C:C + 1, ts(ft, 128)])
                for tb in range(TQ // 512):
                    ps = psa.tile([128, 512], F32, tag="acc", name="ps_f1")
                    for cc in range(NC8):
                        nc.tensor.matmul(ps[:], w_t[:, cc, :],
                                         xs2T[cc][:, ts(tb, 512)],
                                         start=(cc == 0),
                                         stop=(cc == NC8 - 1 and not has_bias))
                    if has_bias:
                        nc.tensor.matmul(ps[:], w_aug[:],
                                         ones2[:, ts(tb, 512)],
                                         start=False, stop=True)
                    nc.scalar.activation(geluT[ft][:, ts(tb, 512)], ps[:], gelu_fn)

        # fc2 + residual
        with tc.tile_pool(name="wk7", bufs=2) as wk7:
            for cb in range(2):
                for qt in range(NQ):
                    ps = psa.tile([128, 512], F32, tag="acc", name="ps_f2")
                    for ft in range(NF):
                        nc.tensor.matmul(ps[:], geluT[ft][:, ts(qt, 128)],
                                         w3_ts[(cb, ft)][:],
                                         start=(ft == 0),
                                         stop=(ft == NF - 1 and not has_bias))
                    if has_bias:
                        nc.tensor.matmul(ps[:], ones_b16[:, ts(qt, 128)],
                                         w3_ts[(cb, "aug")][:],
                                         start=False, stop=True)
                    out_t = wk7.tile([128, 512], F32, tag="out_t", name="out_t")
                    nc.vector.tensor_add(out_t[:], ps[:],
                                         x2_sb[qt][:, ts(cb, 512)])
                    nc.sync.dma_start(out[ts(qt, 128), ts(cb, 512)], out_t[:])


def _dr_pack(W):
    """[C, N] -> flat [128, (C/256)*2*N] DoubleRow stationary layout:
    [p, ci, j, n] = W[(2ci+j)*128 + p, n]."""
    Cr, N = W.shape
    nci = Cr // 256
    Wr = W.reshape(nci, 2, 128, N).transpose(2, 0, 1, 3)   # [128, ci, 2, N]
    return np.ascontiguousarray(Wr.reshape(128, nci * 2 * N))


def host_prep(inputs):
    """Build per-core input maps (all numpy, layout/weight-folding only)."""
    x = np.asarray(inputs["hidden_states"], np.float32)
    w_attn = np.asarray(inputs["w_attn"], np.float32)
    b_attn = np.asarray(inputs["b_attn"], np.float32)
    w_proj = np.asarray(inputs["w_proj"], np.float32)
    b_proj = np.asarray(inputs["b_proj"], np.float32)
    ln1_g = np.asarray(inputs["ln1_g"], np.float32)
    ln1_b = np.asarray(inputs["ln1_b"], np.float32)
    ln2_g = np.asarray(inputs["ln2_g"], np.float32)
    ln2_b = np.asarray(inputs["ln2_b"], np.float32)
    w_fc = np.asarray(inputs["w_fc"], np.float32)
    b_fc = np.asarray(inputs["b_fc"], np.float32)
    w_fc2 = np.asarray(inputs["w_fc2"], np.float32)
    b_fc2 = np.asarray(inputs["b_fc2"], np.float32)

    F8NP = ml_dtypes.float8_e4m3

    W1 = ln1_g[:, None] * w_attn * SW
    bias1 = ln1_b @ w_attn + b_attn
    # K/Q 128-col blocks, interleaved [dt][K,Q]
    kq_blocks = []
    b_blocks = []
    for dt in range(NC8):
        kq_blocks.append(_dr_pack(W1[:, C + dt * 128:C + (dt + 1) * 128]))
        kq_blocks.append(_dr_pack(W1[:, dt * 128:(dt + 1) * 128]))
        b_blocks.append(bias1[C + dt * 128:C + (dt + 1) * 128] * SW)
        b_blocks.append(bias1[dt * 128:(dt + 1) * 128] * SW)
    w1kq = np.stack(kq_blocks).astype(F8NP)
    b1kq = np.stack(b_blocks).astype(ml_dtypes.bfloat16)
    w1v = np.stack([_dr_pack(W1[:, 2 * C + hb * 512:2 * C + (hb + 1) * 512])
                    for hb in range(2)]).astype(F8NP)
    b1v = np.stack([bias1[2 * C + hb * 512:2 * C + (hb + 1) * 512] * SW
                    for hb in range(2)]).astype(ml_dtypes.bfloat16)
    wp2 = np.stack([_dr_pack(w_proj[:, cb * 512:(cb + 1) * 512] * SW)
                    for cb in range(2)]).astype(F8NP)
    bp2 = np.stack([b_proj[cb * 512:(cb + 1) * 512] * (SA * SW)
                    for cb in range(2)]).astype(ml_dtypes.bfloat16)

    W2 = ln2_g[:, None] * w_fc
    bias2 = ln2_b @ w_fc + b_fc
    w2aug = np.concatenate([W2, bias2[None, :]], 0).astype(ml_dtypes.bfloat16)
    w3aug = np.concatenate([w_fc2, b_fc2[None, :]], 0).astype(ml_dtypes.bfloat16)
    has_bias = bool(np.any(bias1) or np.any(bias2) or np.any(b_proj)
                    or np.any(b_fc2))

    in_maps = []
    slices = []
    karr = np.arange(T)
    for c in range(NCORES):
        b, j = c // 2, c % 2
        blockA = np.arange(j * 512, (j + 1) * 512)
        blockB = np.arange(1024 + j * 512, 1536 + j * 512)
        own = np.concatenate([blockA, blockB])
        xq_np = np.ascontiguousarray(x[b][own])
        maskc = np.empty((T, 512), np.float32)
        maskc[:1024] = (karr[:1024, None] <= blockA[None, :])
        maskc[1024:] = (karr[1024:, None] <= blockB[None, :])
        m = {
            "xb": np.ascontiguousarray(x[b]), "xq": xq_np,
            "maskc": maskc.astype(ml_dtypes.bfloat16),
            "w1kq": w1kq, "w1v": w1v, "wp2": wp2,
            "w2aug": w2aug, "w3aug": w3aug,
        }
        if has_bias:
            m.update({"b1kq": b1kq, "b1v": b1v, "bp2": bp2})
        in_maps.append(m)
        slices.append((b, own))
    return in_maps, slices, has_bias


_NC_CACHE = {}


def kernel(**inputs):
    in_maps, slices, has_bias = host_prep(inputs)
    if has_bias not in _NC_CACHE:
        _NC_CACHE[has_bias] = build_program(has_bias=has_bias)
    nc = _NC_CACHE[has_bias]
    res = run_bass_kernel_spmd(nc, in_maps, list(range(NCORES)))
    out = np.empty((B, T, C), np.float32)
    for c, (b, own) in enumerate(slices):
        out[b, own] = res.results[c]["out"]
    return out
